# revision 8
# baseline (speedup 1.0000x reference)
"""Trainium2 Bass kernel for nn_Classifier_87256555586283 (KAN 2-layer MLP).

Math: each kan_linear(x) = silu(x) @ base_w.T + einsum('nig,oig->no', B(x), spline_w*scaler)
where B(x) are 8 cubic B-spline bases on a uniform grid (knots 0.4 apart, t0=-2.2).

Key reformulation: with u = clip(2.5*x + 5.5, 0, 11), the 8 B-spline bases are an
exact linear combination of 11 one-sided cubes phi_s(u) = relu(u - s)^3, s=0..10
(divided-difference identity B3(v) = (1/6) * sum_r (-1)^r C(4,r) relu(v-r)^3; the
s=11 plane vanishes on the clamped domain).  The (8 -> 11) basis transform is folded
into the weights on the host, so on-device each layer is:
  planes: silu(x) plus 11 cheap elementwise planes (relu / square / multiply)
  one fp32 matmul with contraction K = 12*768, tokens on the free dim.

Sharding: pure data-parallel over the flattened 16384 tokens across 8 cores
(2048 tokens/core), weights replicated, no collectives.

Device program per core (all fp32):
  - x^T [768, 2048] staged host-side (feature-major for the matmul contraction).
  - L1 contraction (72 chunks of K=128) split into 3 passes of 24 chunks so pass
    weights (4.6 MB) stay SBUF-resident; partial sums round-trip via DRAM.
  - PSUM: 6 banks = 6 output chunks x 512-token block; 4 token blocks per core.
  - L2 (768 -> 2): same 12 plane types, M=2 matmuls packed 3-per-PSUM-bank via
    tile_position col-groups; col-groups summed with one ones-matmul.
  - ACT table sets: pass0 uses silu set; first Gelu switches to gelu set; L2's
    silu is computed as (y/2)*(1+tanh(y/2)) to stay in the gelu set.
"""

import math

import numpy as np

# problem constants (hardcoded per contract)
B, S, H, L = 32, 512, 768, 2
NTOK = B * S            # 16384
NCORES = 8
TPC = NTOK // NCORES    # 2048 tokens per core
NTB = 512               # token block (PSUM bank = 512 fp32)
NNT = TPC // NTB        # 4
NS = 11                 # relu-cube planes
NIC = H // 128          # 6
NOC = H // 128          # 6

# L1 chunk-type schedule: 3 passes x 4 types ('b' = silu base plane, int = s)
PASS_TYPES = (("b", 0, 1, 2), (3, 4, 5, 6), (7, 8, 9, 10))
L2_TYPES = ("b", 0, 1, 2, 3, 4, 5, 6, 7, 8, 9, 10)

# engine assignment per plane s: (d_engine, e_engine, p_engine)
# d = relu(u-s) [tensor_scalar dual-op], e = (u-s)^2 [ACT Square or d*d],
# p = phi = d*e.  'v' vector, 'g' gpsimd, 'a' scalar/ACT.  d None => s==0, d=u.
PLANE_ENG = {
    0: (None, "a", "v"),
    1: ("g", "a", "v"),
    2: ("g", "a", "v"),
    3: ("g", "a", "v"),
    4: ("g", "a", "g"),
    5: ("g", "a", "v"),
    6: ("g", "a", "g"),
    7: ("g", "a", "v"),
    8: ("v", "a", "g"),
    9: ("v", "a", "v"),
    10: ("v", "v", "g"),
}

_PROGRAM = None


def _basis_transform():
    """(8, 11) matrix C with bases[g] = sum_s C[g, s] * relu(u - s)^3 on [0, 11]."""
    C = np.zeros((8, 12), np.float64)
    for g in range(8):
        for r in range(5):
            C[g, g + r] = ((-1) ** r) * math.comb(4, r) / 6.0
    return C[:, :11]


def _pack_weights(base_w1, spline_w1, scaler1, base_w2, spline_w2, scaler2):
    C = _basis_transform()
    W1p = np.einsum(
        "oig,gs->ois",
        spline_w1.astype(np.float64) * scaler1[..., None].astype(np.float64),
        C,
    ).astype(np.float32)  # (768, 768, 11)
    W2p = np.einsum(
        "oig,gs->ois",
        spline_w2.astype(np.float64) * scaler2[..., None].astype(np.float64),
        C,
    ).astype(np.float32)  # (2, 768, 11)
    b1 = base_w1.astype(np.float32)
    b2 = base_w2.astype(np.float32)

    wpacks = []
    for types in PASS_TYPES:
        blocks = np.empty((128, NIC * len(types), NOC, 128), np.float32)
        for ic in range(NIC):
            isl = slice(ic * 128, (ic + 1) * 128)
            for j, t in enumerate(types):
                k = ic * len(types) + j
                for oc in range(NOC):
                    osl = slice(oc * 128, (oc + 1) * 128)
                    if t == "b":
                        blk = b1[osl, isl].T
                    else:
                        blk = W1p[osl, isl, t].T
                    blocks[:, k, oc, :] = blk
        wpacks.append(np.ascontiguousarray(blocks.reshape(128, -1)))

    # L2 pack: 72 chunks x [128, 2] plus 2 ones-columns for the col-group sum
    w2arr = np.zeros((128, len(L2_TYPES) * NIC * 2 + 2), np.float32)
    for ic in range(NIC):
        isl = slice(ic * 128, (ic + 1) * 128)
        for j, t in enumerate(L2_TYPES):
            k2 = ic * len(L2_TYPES) + j
            blk = b2[:, isl].T if t == "b" else W2p[:, isl, t].T  # [128, 2]
            w2arr[:, k2 * 2 : k2 * 2 + 2] = blk
    for cg in range(3):
        for l in range(2):
            w2arr[32 * cg + l, 144 + l] = 1.0
    return wpacks, w2arr


def _build_program(sim_compat=False):
    import concourse.bass as bass  # noqa: F401
    import concourse.tile as tile
    from concourse import bacc, mybir

    f32 = mybir.dt.float32
    A = mybir.ActivationFunctionType
    OP = mybir.AluOpType

    nc = bacc.Bacc(None, target_bir_lowering=False, debug=False)
    # activation() float biases need registered const APs ([128,1] SBUF)
    for val in [5.5] + [-float(s) for s in range(1, NS)]:
        t = nc.alloc_sbuf_tensor(f"constb-{val}", [128, 1], f32)
        nc.gpsimd.memset(t.ap(), val)
        nc.const_aps.aps[(f32, val)] = t.ap()
    nc.all_engine_barrier()

    xT_d = nc.dram_tensor("xT", [H, TPC], f32, kind="ExternalInput")
    w_d = [
        nc.dram_tensor(f"w{p}", [128, 24 * NOC * 128], f32, kind="ExternalInput")
        for p in range(3)
    ]
    wl2_d = nc.dram_tensor("wl2", [128, 146], f32, kind="ExternalInput")
    out_d = nc.dram_tensor("outT", [L, TPC], f32, kind="ExternalOutput")

    with tile.TileContext(nc) as tc:
        from contextlib import ExitStack

        with ExitStack() as ctx:
            dram = ctx.enter_context(tc.tile_pool(name="dram", bufs=1, space="DRAM"))
            partial0 = dram.tile([H, TPC], f32, name="partial0")
            partial1 = dram.tile([H, TPC], f32, name="partial1")
            u_dr = dram.tile([H, TPC], f32, name="u_dr")
            y_dr = dram.tile([H, TPC], f32, name="y_dr")

            xpool = ctx.enter_context(tc.tile_pool(name="xTp", bufs=1))
            xts = []
            for ic in range(NIC):
                xt = xpool.tile([128, TPC], f32, name=f"xt{ic}", tag=f"xt{ic}")
                nc.sync.dma_start(xt[:], xT_d[ic * 128 : (ic + 1) * 128, :])
                xts.append(xt)

            w2pool = ctx.enter_context(tc.tile_pool(name="w2p", bufs=1))
            w2sb = w2pool.tile([128, 146], f32, name="w2sb")
            nc.sync.dma_start(w2sb[:], wl2_d[:])

            psum = ctx.enter_context(
                tc.tile_pool(name="psum", bufs=1, space="PSUM")
            )
            l1ps = [
                psum.tile([128, NTB], f32, name=f"l1ps{oc}", tag=f"l1ps{oc}")
                for oc in range(NOC)
            ]
            l2ps = psum.tile([128, NTB], f32, name="l2ps", tag="l2ps")
            fps = psum.tile([128, NTB], f32, name="fps", tag="fps")
            nc.vector.memset(l2ps[:], 0.0)

            sm = ctx.enter_context(tc.tile_pool(name="sm", bufs=1))

            eng = {"v": nc.vector, "g": nc.gpsimd}

            def build_plane(u, s):
                d_e, e_e, p_e = PLANE_ENG[s]
                if d_e is None:
                    d = u
                else:
                    d = sm.tile([128, NTB], f32, name=f"d{s}", tag="d", bufs=3)
                    eng[d_e].tensor_scalar(
                        d[:], u[:], -float(s), 0.0, OP.add, OP.max
                    )
                e = sm.tile([128, NTB], f32, name=f"e{s}", tag="e", bufs=3)
                if e_e == "a":
                    nc.scalar.activation(
                        e[:], u[:], A.Square, bias=-float(s), scale=1.0
                    )
                else:
                    eng[e_e].tensor_tensor(e[:], d[:], d[:], OP.mult)
                ph = sm.tile([128, NTB], f32, name=f"ph{s}", tag="phi", bufs=5)
                eng[p_e].tensor_tensor(ph[:], d[:], e[:], OP.mult)
                return ph

            # ---------------- Layer 1: 3 passes over K ----------------
            for p, types in enumerate(PASS_TYPES):
                with tc.tile_pool(name=f"w{p}pool", bufs=1) as wp:
                    wsb = wp.tile([128, 24 * NOC * 128], f32, name=f"wsb{p}")
                    blk = 4 * NOC * 128  # one ic worth of chunks
                    for ic in range(NIC):
                        nc.sync.dma_start(
                            wsb[:, ic * blk : (ic + 1) * blk],
                            w_d[p][:, ic * blk : (ic + 1) * blk],
                        )
                    for nt in range(NNT):
                        tsl = slice(nt * NTB, (nt + 1) * NTB)
                        for ic in range(NIC):
                            xs = xts[ic][:, tsl]
                            if p == 0:
                                ur = sm.tile(
                                    [128, NTB], f32, name="ur", tag="u", bufs=4
                                )
                                nc.scalar.activation(
                                    ur[:], xs, A.Relu, bias=5.5, scale=2.5
                                )
                                u = sm.tile(
                                    [128, NTB], f32, name="u", tag="u", bufs=4
                                )
                                nc.vector.tensor_scalar(
                                    u[:], ur[:], 11.0, None, OP.min
                                )
                                nc.sync.dma_start(
                                    u_dr[ic * 128 : (ic + 1) * 128, tsl], u[:]
                                )
                            else:
                                u = sm.tile(
                                    [128, NTB], f32, name="uld", tag="uld", bufs=3
                                )
                                nc.sync.dma_start(
                                    u[:], u_dr[ic * 128 : (ic + 1) * 128, tsl]
                                )
                            for j, t in enumerate(types):
                                k = ic * 4 + j
                                if t == "b":
                                    # silu(x) = x * sigmoid(x)
                                    sg = sm.tile(
                                        [128, NTB], f32, name="sg", tag="th",
                                        bufs=2,
                                    )
                                    nc.scalar.activation(sg[:], xs, A.Sigmoid)
                                    pl = sm.tile(
                                        [128, NTB], f32, name="sil", tag="phi",
                                        bufs=5,
                                    )
                                    nc.vector.tensor_tensor(
                                        pl[:], xs, sg[:], OP.mult
                                    )
                                else:
                                    pl = build_plane(u, t)
                                for oc in range(NOC):
                                    nc.tensor.matmul(
                                        l1ps[oc][:],
                                        wsb[:, (k * NOC + oc) * 128 : (k * NOC + oc + 1) * 128],
                                        pl[:],
                                        start=(k == 0),
                                        stop=(k == 23),
                                    )
                        # end of pass for this token block: drain PSUM
                        for oc in range(NOC):
                            osl = slice(oc * 128, (oc + 1) * 128)
                            if p == 0:
                                st = sm.tile(
                                    [128, NTB], f32, name="evac", tag="evac", bufs=3
                                )
                                nc.scalar.copy(st[:], l1ps[oc][:])
                                nc.sync.dma_start(partial0[osl, tsl], st[:])
                            elif p == 1:
                                ld = sm.tile(
                                    [128, NTB], f32, name="pld", tag="pld", bufs=3
                                )
                                nc.sync.dma_start(ld[:], partial0[osl, tsl])
                                st = sm.tile(
                                    [128, NTB], f32, name="evac", tag="evac", bufs=3
                                )
                                nc.vector.scalar_tensor_tensor(
                                    st[:], l1ps[oc][:], 0.0, ld[:], OP.add, OP.add
                                )
                                nc.sync.dma_start(partial1[osl, tsl], st[:])
                            else:
                                ld = sm.tile(
                                    [128, NTB], f32, name="pld", tag="pld", bufs=3
                                )
                                nc.sync.dma_start(ld[:], partial1[osl, tsl])
                                pre = sm.tile(
                                    [128, NTB], f32, name="pre", tag="pre", bufs=2
                                )
                                nc.vector.scalar_tensor_tensor(
                                    pre[:], l1ps[oc][:], 0.0, ld[:], OP.add, OP.add
                                )
                                y = sm.tile(
                                    [128, NTB], f32, name="y", tag="y", bufs=2
                                )
                                gelu_f = A.Tanh if sim_compat else A.Gelu
                                nc.scalar.activation(y[:], pre[:], gelu_f)
                                nc.sync.dma_start(y_dr[osl, tsl], y[:])

            # ---------------- Layer 2 ----------------
            for nt in range(NNT):
                tsl = slice(nt * NTB, (nt + 1) * NTB)
                for ic in range(NIC):
                    yt = sm.tile([128, NTB], f32, name="yld", tag="yld", bufs=3)
                    nc.sync.dma_start(yt[:], y_dr[ic * 128 : (ic + 1) * 128, tsl])
                    ur = sm.tile([128, NTB], f32, name="u2r", tag="u", bufs=4)
                    nc.scalar.activation(ur[:], yt[:], A.Relu, bias=5.5, scale=2.5)
                    u = sm.tile([128, NTB], f32, name="u2", tag="u", bufs=4)
                    nc.vector.tensor_scalar(u[:], ur[:], 11.0, None, OP.min)
                    for j, t in enumerate(L2_TYPES):
                        k2 = ic * 12 + j
                        cg = k2 % 3
                        if t == "b":
                            # silu(y) = (y/2) * (1 + tanh(y/2)) — stays in gelu set
                            th = sm.tile(
                                [128, NTB], f32, name="th", tag="th", bufs=2
                            )
                            nc.scalar.activation(th[:], yt[:], A.Tanh, scale=0.5)
                            t1 = sm.tile(
                                [128, NTB], f32, name="t1", tag="th", bufs=2
                            )
                            nc.vector.tensor_scalar(
                                t1[:], th[:], 0.5, 0.5, OP.mult, OP.add
                            )
                            pl = sm.tile(
                                [128, NTB], f32, name="sil2", tag="phi", bufs=5
                            )
                            nc.gpsimd.tensor_tensor(pl[:], yt[:], t1[:], OP.mult)
                        else:
                            pl = build_plane(u, t)
                        nc.tensor.matmul(
                            l2ps[32 * cg : 32 * cg + 2, :],
                            w2sb[:, k2 * 2 : k2 * 2 + 2],
                            pl[:],
                            start=(k2 < 3),
                            stop=(k2 >= 69),
                            tile_position=(0, 32 * cg),
                        )
                cp = sm.tile([128, NTB], f32, name="cp", tag="cp", bufs=2)
                nc.vector.tensor_copy(cp[:], l2ps[:])
                nc.tensor.matmul(
                    fps[0:L, :], w2sb[:, 144:146], cp[:], start=True, stop=True
                )
                ob = sm.tile([L, NTB], f32, name="ob", tag="ob", bufs=2)
                nc.vector.tensor_copy(ob[:], fps[0:L, :])
                nc.sync.dma_start(out_d[:, tsl], ob[:])

    nc.compile()
    return nc


def _get_program():
    global _PROGRAM
    if _PROGRAM is None:
        _PROGRAM = _build_program()
    return _PROGRAM


def run(hidden, base_w1, spline_w1, scaler1, base_w2, spline_w2, scaler2, **kw):
    """Builds inputs, runs the SPMD kernel on 8 cores. Returns (output, results)."""
    from concourse.bass_utils import run_bass_kernel_spmd

    nc = _get_program()
    x = np.ascontiguousarray(
        np.asarray(hidden, dtype=np.float32).reshape(NTOK, H)
    )
    wpacks, w2arr = _pack_weights(
        np.asarray(base_w1), np.asarray(spline_w1), np.asarray(scaler1),
        np.asarray(base_w2), np.asarray(spline_w2), np.asarray(scaler2),
    )
    in_maps = []
    for c in range(NCORES):
        xT = np.ascontiguousarray(x[c * TPC : (c + 1) * TPC].T)
        in_maps.append(
            {"xT": xT, "w0": wpacks[0], "w1": wpacks[1], "w2": wpacks[2],
             "wl2": w2arr}
        )
    res = run_bass_kernel_spmd(nc, in_maps, list(range(NCORES)), **kw)
    outs = [r["outT"].T for r in res.results]  # each (2048, 2)
    out = np.concatenate(outs, axis=0).reshape(B, S, L).astype(np.float32)
    return out, res


def kernel(**inputs):
    out, _ = run(**inputs)
    return out


# revision 15
# speedup vs baseline: 1.8183x; 1.8183x over previous
"""Trainium2 Bass kernel for nn_Classifier_87256555586283 (KAN 2-layer MLP).

Math: each kan_linear(x) = silu(x) @ base_w.T + einsum('nig,oig->no', B(x), spline_w*scaler)
where B(x) are 8 cubic B-spline bases on a uniform grid (knots 0.4 apart, t0=-2.2).

Reformulation: with u = clip(2.5*x + 5.5, 0, 11), the 8 bases are an exact linear
combination of 11 one-sided cubes phi_s(u) = relu(u - s)^3, s=0..10 (divided
differences of B3; the s=11 plane vanishes on the clamped domain).  The 8->11
transform is folded into the weights host-side, so each layer becomes 12 cheap
elementwise planes (silu + 11 cubes) feeding one matmul with K = 12*768.

Matmul precision/speed: fp32 matmuls cost 4 PE-cycles/row on TRN2 (2-pass).
Instead we use an exact-ish split: planes P = Ph + Pl and weights W = Wh + Wl
(fp16 hi + fp16 residual; fp16 products are exact in fp32 PSUM), computing
  W@P ~= Wh@Ph + Wh@Pl + Wl@Ph      (3 fp16 matmuls at 1 cyc/row)
measured end-to-end error 1.9e-4 relative (vs 1.1e-4 for full fp32).

Sharding: data-parallel over the 16384 tokens across 8 cores, weights
replicated, no collectives.  x^T is staged host-side (feature-major).

Structure per core: L1 contraction (72 chunks of K=128) split into 3 passes of
24 chunks (pass weights SBUF-resident; partial sums via DRAM).  PSUM: 6 banks =
6 output chunks x 512-token block, 4 token blocks.  L2 (768->2): M=2 matmuls
packed 3-per-bank via tile_position col groups, summed with a ones-matmul.
ACT table sets: sigmoid set (Sigmoid for silu) then gelu set (Gelu + Tanh for
L2's silu) — two loads total.  GPSIMD only runs plain tensor_tensor (its
dual-op tensor_scalar measures 7.6us/op — never use).
"""

import math

import numpy as np

# problem constants (hardcoded per contract)
B, S, H, L = 32, 512, 768, 2
NTOK = B * S            # 16384
NCORES = 8
TPC = NTOK // NCORES    # 2048 tokens per core
NTB = 512               # token block (PSUM bank = 512 fp32)
NNT = TPC // NTB        # 4
NS = 11                 # relu-cube feature planes
NIC = H // 128          # 6
NOC = H // 128          # 6

# L1 chunk-type schedule: 3 passes x 4 types ('b' = silu base plane, int = s)
PASS_TYPES = (("b", 0, 1, 2), (3, 4, 5, 6), (7, 8, 9, 10))
L2_TYPES = ("b", 0, 1, 2, 3, 4, 5, 6, 7, 8, 9, 10)

# engine split for the phi planes: d = relu(u-s) [DVE tensor_scalar dual-op],
# e = (u-s)^2 [ACT Square], p32 = d*e [DVE or GPSIMD tensor_tensor].
P_ENG = {0: "v", 1: "g", 2: "v", 3: "g", 4: "v", 5: "g", 6: "v", 7: "g",
         8: "v", 9: "g", 10: "v"}
# fp16 pair production: hi = cast (ACT copy or DVE tensor_scalar), lo = sub
# (DVE or GPSIMD tensor_tensor) — rotate to balance.
CAST_ENG = {0: "a", 1: "v", 2: "a", 3: "v", 4: "a", 5: "v", 6: "a", 7: "v",
            8: "a", 9: "v", 10: "a", "b": "v"}
SUB_ENG = {0: "v", 1: "g", 2: "g", 3: "v", 4: "g", 5: "v", 6: "g", 7: "v",
           8: "g", 9: "v", 10: "g", "b": "g"}

_PROGRAM = None


def _basis_transform():
    """(8, 11) matrix C with bases[g] = sum_s C[g, s] * relu(u - s)^3 on [0, 11]."""
    C = np.zeros((8, 12), np.float64)
    for g in range(8):
        for r in range(5):
            C[g, g + r] = ((-1) ** r) * math.comb(4, r) / 6.0
    return C[:, :11]


def _split16(a):
    hi = a.astype(np.float16)
    lo = (a.astype(np.float32) - hi.astype(np.float32)).astype(np.float16)
    return hi, lo


def _pack_weights(base_w1, spline_w1, scaler1, base_w2, spline_w2, scaler2):
    C = _basis_transform()
    W1p = np.einsum(
        "oig,gs->ois",
        spline_w1.astype(np.float64) * scaler1[..., None].astype(np.float64),
        C,
    ).astype(np.float32)  # (768, 768, 11)
    W2p = np.einsum(
        "oig,gs->ois",
        spline_w2.astype(np.float64) * scaler2[..., None].astype(np.float64),
        C,
    ).astype(np.float32)  # (2, 768, 11)
    b1 = base_w1.astype(np.float32)
    b2 = base_w2.astype(np.float32)

    wpacks = []
    for types in PASS_TYPES:
        blocks = np.empty((128, NIC * len(types), NOC, 128), np.float32)
        for ic in range(NIC):
            isl = slice(ic * 128, (ic + 1) * 128)
            for j, t in enumerate(types):
                k = ic * len(types) + j
                for oc in range(NOC):
                    osl = slice(oc * 128, (oc + 1) * 128)
                    blk = b1[osl, isl].T if t == "b" else W1p[osl, isl, t].T
                    blocks[:, k, oc, :] = blk
        hi, lo = _split16(blocks.reshape(128, -1))
        wpacks.append((np.ascontiguousarray(hi), np.ascontiguousarray(lo)))

    # L2 pack: 72 chunks x [128, 2] plus 2 ones-columns for the col-group sum
    w2arr = np.zeros((128, len(L2_TYPES) * NIC * 2 + 2), np.float32)
    for ic in range(NIC):
        isl = slice(ic * 128, (ic + 1) * 128)
        for j, t in enumerate(L2_TYPES):
            k2 = ic * len(L2_TYPES) + j
            blk = b2[:, isl].T if t == "b" else W2p[:, isl, t].T  # [128, 2]
            w2arr[:, k2 * 2 : k2 * 2 + 2] = blk
    w2ones = np.zeros((128, 2), np.float32)
    for cg in range(3):
        for l in range(2):
            w2ones[32 * cg + l, l] = 1.0
    w2h, w2l = _split16(w2arr)
    return wpacks, (np.ascontiguousarray(w2h), np.ascontiguousarray(w2l), w2ones)


def _build_program(sim_compat=False):
    import concourse.bass as bass  # noqa: F401
    import concourse.tile as tile
    from concourse import bacc, mybir

    f32 = mybir.dt.float32
    f16 = mybir.dt.float16
    A = mybir.ActivationFunctionType
    OP = mybir.AluOpType

    nc = bacc.Bacc(None, target_bir_lowering=False, debug=False)
    # activation() float biases need registered const APs ([128,1] SBUF)
    for val in [5.5] + [-float(s) for s in range(1, NS)]:
        t = nc.alloc_sbuf_tensor(f"constb-{val}", [128, 1], f32)
        nc.gpsimd.memset(t.ap(), val)
        nc.const_aps.aps[(f32, val)] = t.ap()
    nc.all_engine_barrier()

    xT_d = nc.dram_tensor("xT", [H, TPC], f32, kind="ExternalInput")
    wh_d = [
        nc.dram_tensor(f"w{p}h", [128, 24 * NOC * 128], f16, kind="ExternalInput")
        for p in range(3)
    ]
    wl_d = [
        nc.dram_tensor(f"w{p}l", [128, 24 * NOC * 128], f16, kind="ExternalInput")
        for p in range(3)
    ]
    w2h_d = nc.dram_tensor("wl2h", [128, 146], f16, kind="ExternalInput")
    w2l_d = nc.dram_tensor("wl2l", [128, 146], f16, kind="ExternalInput")
    w2o_d = nc.dram_tensor("wl2o", [128, 2], f32, kind="ExternalInput")
    out_d = nc.dram_tensor("outT", [L, TPC], f32, kind="ExternalOutput")

    with tile.TileContext(nc) as tc:
        from contextlib import ExitStack

        with ExitStack() as ctx:
            dram = ctx.enter_context(tc.tile_pool(name="dram", bufs=1, space="DRAM"))
            partial0 = dram.tile([H, TPC], f32, name="partial0")
            partial1 = dram.tile([H, TPC], f32, name="partial1")
            u_dr = dram.tile([H, TPC], f32, name="u_dr")
            y_dr = dram.tile([H, TPC], f32, name="y_dr")

            xpool = ctx.enter_context(tc.tile_pool(name="xTp", bufs=1))
            xts = []
            for ic in range(NIC):
                xt = xpool.tile([128, TPC], f32, name=f"xt{ic}", tag=f"xt{ic}")
                nc.sync.dma_start(xt[:], xT_d[ic * 128 : (ic + 1) * 128, :])
                xts.append(xt)

            w2pool = ctx.enter_context(tc.tile_pool(name="w2p", bufs=1))
            w2h = w2pool.tile([128, 146], f16, name="w2h_sb")
            w2l = w2pool.tile([128, 146], f16, name="w2l_sb")
            w2o = w2pool.tile([128, 2], f32, name="w2o_sb")
            nc.sync.dma_start(w2h[:], w2h_d[:])
            nc.sync.dma_start(w2l[:], w2l_d[:])
            nc.sync.dma_start(w2o[:], w2o_d[:])

            psum = ctx.enter_context(tc.tile_pool(name="psum", bufs=1, space="PSUM"))
            l1ps = [
                psum.tile([128, NTB], f32, name=f"l1ps{oc}", tag=f"l1ps{oc}")
                for oc in range(NOC)
            ]
            l2ps = psum.tile([128, NTB], f32, name="l2ps", tag="l2ps")
            fps = psum.tile([128, NTB], f32, name="fps", tag="fps")
            nc.vector.memset(l2ps[:], 0.0)

            sm = ctx.enter_context(tc.tile_pool(name="sm", bufs=1))
            eng = {"v": nc.vector, "g": nc.gpsimd}

            def make_pair(src32, key, tag):
                """fp16 (hi, lo) pair of an fp32 tile."""
                ph = sm.tile([128, NTB], f16, name=f"ph{key}", tag=f"{tag}h", bufs=4)
                if CAST_ENG[key] == "a":
                    nc.scalar.copy(ph[:], src32[:])
                else:
                    nc.vector.tensor_scalar(ph[:], src32[:], 0.0, None, OP.add)
                pl = sm.tile([128, NTB], f16, name=f"pl{key}", tag=f"{tag}l", bufs=4)
                eng[SUB_ENG[key]].tensor_tensor(pl[:], src32[:], ph[:], OP.subtract)
                return ph, pl

            def build_plane(u, s):
                if s == 0:
                    d = u
                else:
                    d = sm.tile([128, NTB], f32, name=f"d{s}", tag="d", bufs=3)
                    nc.vector.tensor_scalar(
                        d[:], u[:], -float(s), 0.0, OP.add, OP.max
                    )
                e = sm.tile([128, NTB], f32, name=f"e{s}", tag="e", bufs=3)
                nc.scalar.activation(e[:], u[:], A.Square, bias=-float(s), scale=1.0)
                p32 = sm.tile([128, NTB], f32, name=f"p32{s}", tag="p32", bufs=3)
                eng[P_ENG[s]].tensor_tensor(p32[:], d[:], e[:], OP.mult)
                return make_pair(p32, s, "phi")

            def mm3(ps_ap, wh_ap, wl_ap, ph, pl, start, stop, tile_position=None):
                nc.tensor.matmul(ps_ap, wh_ap, ph[:], start=start, stop=False,
                                 tile_position=tile_position)
                nc.tensor.matmul(ps_ap, wh_ap, pl[:], start=False, stop=False,
                                 tile_position=tile_position)
                nc.tensor.matmul(ps_ap, wl_ap, ph[:], start=False, stop=stop,
                                 tile_position=tile_position)

            # ---------------- Layer 1: 3 passes over K ----------------
            for p, types in enumerate(PASS_TYPES):
                with tc.tile_pool(name=f"w{p}pool", bufs=1) as wp:
                    wsbh = wp.tile([128, 24 * NOC * 128], f16, name=f"wsbh{p}")
                    wsbl = wp.tile([128, 24 * NOC * 128], f16, name=f"wsbl{p}")
                    blk = 4 * NOC * 128
                    for ic in range(NIC):
                        csl = slice(ic * blk, (ic + 1) * blk)
                        nc.sync.dma_start(wsbh[:, csl], wh_d[p][:, csl])
                        nc.sync.dma_start(wsbl[:, csl], wl_d[p][:, csl])
                    for nt in range(NNT):
                        tsl = slice(nt * NTB, (nt + 1) * NTB)
                        for ic in range(NIC):
                            xs = xts[ic][:, tsl]
                            if p == 0:
                                ur = sm.tile([128, NTB], f32, name="ur", tag="u",
                                             bufs=4)
                                nc.scalar.activation(
                                    ur[:], xs, A.Relu, bias=5.5, scale=2.5
                                )
                                u = sm.tile([128, NTB], f32, name="u", tag="u",
                                            bufs=4)
                                nc.vector.tensor_scalar(
                                    u[:], ur[:], 11.0, None, OP.min
                                )
                                nc.sync.dma_start(
                                    u_dr[ic * 128 : (ic + 1) * 128, tsl], u[:]
                                )
                            else:
                                u = sm.tile([128, NTB], f32, name="uld", tag="uld",
                                            bufs=3)
                                nc.sync.dma_start(
                                    u[:], u_dr[ic * 128 : (ic + 1) * 128, tsl]
                                )
                            for j, t in enumerate(types):
                                k = ic * 4 + j
                                if t == "b":
                                    sg = sm.tile([128, NTB], f32, name="sg",
                                                 tag="th", bufs=2)
                                    nc.scalar.activation(sg[:], xs, A.Sigmoid)
                                    sil = sm.tile([128, NTB], f32, name="sil",
                                                  tag="p32", bufs=3)
                                    nc.vector.tensor_tensor(
                                        sil[:], xs, sg[:], OP.mult
                                    )
                                    ph, pl = make_pair(sil, "b", "phi")
                                else:
                                    ph, pl = build_plane(u, t)
                                for oc in range(NOC):
                                    wof = slice((k * NOC + oc) * 128,
                                                (k * NOC + oc + 1) * 128)
                                    mm3(l1ps[oc][:], wsbh[:, wof], wsbl[:, wof],
                                        ph, pl, start=(k == 0), stop=(k == 23))
                        # end of pass for this token block: drain PSUM
                        for oc in range(NOC):
                            osl = slice(oc * 128, (oc + 1) * 128)
                            if p == 0:
                                st = sm.tile([128, NTB], f32, name="evac",
                                             tag="evac", bufs=3)
                                nc.scalar.copy(st[:], l1ps[oc][:])
                                nc.sync.dma_start(partial0[osl, tsl], st[:])
                            elif p == 1:
                                ld = sm.tile([128, NTB], f32, name="pld",
                                             tag="pld", bufs=3)
                                nc.sync.dma_start(ld[:], partial0[osl, tsl])
                                st = sm.tile([128, NTB], f32, name="evac",
                                             tag="evac", bufs=3)
                                nc.vector.scalar_tensor_tensor(
                                    st[:], l1ps[oc][:], 0.0, ld[:], OP.add, OP.add
                                )
                                nc.sync.dma_start(partial1[osl, tsl], st[:])
                            else:
                                ld = sm.tile([128, NTB], f32, name="pld",
                                             tag="pld", bufs=3)
                                nc.sync.dma_start(ld[:], partial1[osl, tsl])
                                pre = sm.tile([128, NTB], f32, name="pre",
                                              tag="pre", bufs=2)
                                nc.vector.scalar_tensor_tensor(
                                    pre[:], l1ps[oc][:], 0.0, ld[:], OP.add, OP.add
                                )
                                y = sm.tile([128, NTB], f32, name="y", tag="y",
                                            bufs=2)
                                gelu_f = A.Tanh if sim_compat else A.Gelu
                                nc.scalar.activation(y[:], pre[:], gelu_f)
                                nc.sync.dma_start(y_dr[osl, tsl], y[:])

            # ---------------- Layer 2 ----------------
            for nt in range(NNT):
                tsl = slice(nt * NTB, (nt + 1) * NTB)
                for ic in range(NIC):
                    yt = sm.tile([128, NTB], f32, name="yld", tag="yld", bufs=3)
                    nc.sync.dma_start(yt[:], y_dr[ic * 128 : (ic + 1) * 128, tsl])
                    ur = sm.tile([128, NTB], f32, name="u2r", tag="u", bufs=4)
                    nc.scalar.activation(ur[:], yt[:], A.Relu, bias=5.5, scale=2.5)
                    u = sm.tile([128, NTB], f32, name="u2", tag="u", bufs=4)
                    nc.vector.tensor_scalar(u[:], ur[:], 11.0, None, OP.min)
                    for j, t in enumerate(L2_TYPES):
                        k2 = ic * 12 + j
                        cg = k2 % 3
                        if t == "b":
                            # silu(y) = y * (0.5 + 0.5*tanh(y/2)) — gelu set
                            th = sm.tile([128, NTB], f32, name="th", tag="th",
                                         bufs=2)
                            nc.scalar.activation(th[:], yt[:], A.Tanh, scale=0.5)
                            t1 = sm.tile([128, NTB], f32, name="t1", tag="th",
                                         bufs=2)
                            nc.vector.tensor_scalar(
                                t1[:], th[:], 0.5, 0.5, OP.mult, OP.add
                            )
                            sil = sm.tile([128, NTB], f32, name="sil2", tag="p32",
                                          bufs=3)
                            nc.gpsimd.tensor_tensor(sil[:], yt[:], t1[:], OP.mult)
                            ph, pl = make_pair(sil, "b", "phi")
                        else:
                            ph, pl = build_plane(u, t)
                        mm3(l2ps[32 * cg : 32 * cg + 2, :],
                            w2h[:, k2 * 2 : k2 * 2 + 2],
                            w2l[:, k2 * 2 : k2 * 2 + 2],
                            ph, pl, start=(k2 < 3), stop=(k2 >= 69),
                            tile_position=(0, 32 * cg))
                cp = sm.tile([128, NTB], f32, name="cp", tag="cp", bufs=2)
                nc.vector.tensor_copy(cp[:], l2ps[:])
                nc.tensor.matmul(fps[0:L, :], w2o[:], cp[:],
                                 start=True, stop=True)
                ob = sm.tile([L, NTB], f32, name="ob", tag="ob", bufs=2)
                nc.vector.tensor_copy(ob[:], fps[0:L, :])
                nc.sync.dma_start(out_d[:, tsl], ob[:])

    nc.compile()
    return nc


def _get_program():
    global _PROGRAM
    if _PROGRAM is None:
        _PROGRAM = _build_program()
    return _PROGRAM


def run(hidden, base_w1, spline_w1, scaler1, base_w2, spline_w2, scaler2, **kw):
    """Builds inputs, runs the SPMD kernel on 8 cores. Returns (output, results)."""
    from concourse.bass_utils import run_bass_kernel_spmd

    nc = _get_program()
    x = np.ascontiguousarray(np.asarray(hidden, dtype=np.float32).reshape(NTOK, H))
    wpacks, w2pair = _pack_weights(
        np.asarray(base_w1), np.asarray(spline_w1), np.asarray(scaler1),
        np.asarray(base_w2), np.asarray(spline_w2), np.asarray(scaler2),
    )
    in_maps = []
    for c in range(NCORES):
        xT = np.ascontiguousarray(x[c * TPC : (c + 1) * TPC].T)
        m = {"xT": xT, "wl2h": w2pair[0], "wl2l": w2pair[1], "wl2o": w2pair[2]}
        for p in range(3):
            m[f"w{p}h"], m[f"w{p}l"] = wpacks[p]
        in_maps.append(m)
    res = run_bass_kernel_spmd(nc, in_maps, list(range(NCORES)), **kw)
    outs = [r["outT"].T for r in res.results]  # each (2048, 2)
    out = np.concatenate(outs, axis=0).reshape(B, S, L).astype(np.float32)
    return out, res


def kernel(**inputs):
    out, _ = run(**inputs)
    return out
